# revision 13
# baseline (speedup 1.0000x reference)
"""Multi-head attention (B=8, T=2048, D=512, H=8) on 8 TRN2 NeuronCores.

Sharding: data-parallel over batch - one batch element per core, no
collectives. Host-side prep: transpose x inputs to [D, T], cast matmul
operands to bf16, pass (1 - mask)^T chunk-major; transpose per-core y^T
back to [T, D].

Per-core algorithm (v2, "row-tiled transposed flash"):
  P1: Q^T = Wq x^T and K^T = Wk x^T, both as 4x [128, T] feature-major
      tiles (two heads per tile, NO padding).  V = x Wv^T with a ones
      column per head (denominator trick), chunk-major vaug tiles.
  P2: heads processed in PAIRS (2j, 2j+1).  Per (q-block of 512, pair,
      t2-chunk c):
        S_A^T = K_A^T.T @ Q_A^T   (K=64, PE rows 0-63)   } row-tiled,
        S_B^T = K_B^T.T @ Q_B^T   (K=64, PE rows 64-127) } concurrent
        both into ONE [128, 1024] PSUM tile (A: cols 0-511, B: 512-1023)
        P_raw = exp(S/8)          one ACTIVATE, N=1024
        P     = P_raw * maskbar   one tensor_mul vs a duplicated mask
                                  tile (1/3 of chunks on GpSimd)
        O_aug^T += Vaug_h.T @ P   (M=65; row 64 = softmax denominator)
      epilogue per (qb, pair): denom rows -> split recip -> DRAM-bounce
      broadcast -> normalize straight into o2 (bf16).
  P3: y^T = Wo^T.T @ O^T (+bo) per q-block, interleaved into the next
      q-block's attention stream.

ScalarE exp (256 ACTIVATEs @ ~1.2us) is the roofline; everything else
is scheduled to hide underneath it.
"""

import numpy as np
import ml_dtypes

B, T, FDIM, H = 8, 2048, 512, 8
DK = FDIM // H          # 64
NFT = FDIM // 128       # 4 feature tiles
NCH = T // 128          # 16 t2-chunks
NQB = 4                 # q blocks
QBS = T // NQB          # 512
N_CORES = 8

BF16 = ml_dtypes.bfloat16

_cache = {}


def _build_nc():
    import concourse.bass as bass
    import concourse.mybir as mybir
    from concourse import bacc, tile

    f32 = mybir.dt.float32
    bf16 = mybir.dt.bfloat16
    Exp = mybir.ActivationFunctionType.Exp
    Alu = mybir.AluOpType

    nc = bacc.Bacc("TRN2", target_bir_lowering=False, debug=False,
                   num_devices=N_CORES)

    # DRAM I/O (per-core shard shapes)
    xqT = nc.dram_tensor("xqT", [FDIM, T], bf16, kind="ExternalInput")
    xkT = nc.dram_tensor("xkT", [FDIM, T], bf16, kind="ExternalInput")
    xvT = nc.dram_tensor("xvT", [FDIM, T], bf16, kind="ExternalInput")
    wqT = nc.dram_tensor("wqT", [FDIM, FDIM], bf16, kind="ExternalInput")
    wkT = nc.dram_tensor("wkT", [FDIM, FDIM], bf16, kind="ExternalInput")
    wvT = nc.dram_tensor("wvT", [FDIM, FDIM], bf16, kind="ExternalInput")
    woT = nc.dram_tensor("woT", [FDIM, FDIM], bf16, kind="ExternalInput")
    bq = nc.dram_tensor("bq", [FDIM], f32, kind="ExternalInput")
    bk = nc.dram_tensor("bk", [FDIM], f32, kind="ExternalInput")
    bv = nc.dram_tensor("bv", [FDIM], f32, kind="ExternalInput")
    bo = nc.dram_tensor("bo", [FDIM], f32, kind="ExternalInput")
    mbar = nc.dram_tensor("mbar", [NCH, 128, T], bf16, kind="ExternalInput")
    yT = nc.dram_tensor("yT", [FDIM, T], f32, kind="ExternalOutput")
    # DRAM bounce rows for partition-broadcasting softmax reciprocals
    rscratch = nc.dram_tensor("rscratch", [NQB * H, QBS], f32)

    with tile.TileContext(nc) as tc:
        with (
            tc.tile_pool(name="consts", bufs=1) as consts,
            tc.tile_pool(name="qt", bufs=1) as qt_pool,
            tc.tile_pool(name="kt", bufs=1) as kt_pool,
            tc.tile_pool(name="vaug", bufs=1) as vaug_pool,
            tc.tile_pool(name="osb", bufs=1) as osb_pool,
        ):
            # ---- consts: weights + biases ----
            # One [128, 4*512] tile per weight matrix -> ONE DMA instruction
            # each (the Sync engine issues descriptors at ~650ns apiece, so
            # instruction count, not bytes, sets the head latency).
            wq_sb = consts.tile([128, 4 * FDIM], bf16, tag="wq", name="wq")
            wk_sb = consts.tile([128, 4 * FDIM], bf16, tag="wk", name="wk")
            wv_sb = consts.tile([128, 4 * FDIM], bf16, tag="wv", name="wv")
            wo_sb = consts.tile([128, 4 * FDIM], bf16, tag="wo", name="wo")

            def w_dma(dst, src):
                nc.sync.dma_start(
                    out=dst[:].rearrange("p (f d) -> p f d", d=FDIM),
                    in_=src.ap().rearrange("(f p) d -> p f d", p=128))

            # wk/wq first: the K/Q projections head the critical path.
            # wo is DMA'd much later (first needed ~100us in).
            w_dma(wk_sb, wkT)
            w_dma(wq_sb, wqT)
            w_dma(wv_sb, wvT)

            bq_sb = consts.tile([128, NFT], f32, tag="bq", name="bq")
            bk_sb = consts.tile([128, NFT], f32, tag="bk", name="bk")
            bo_sb = consts.tile([128, NFT], f32, tag="bo", name="bo")
            for b_dram, b_t in ((bq, bq_sb), (bk, bk_sb), (bo, bo_sb)):
                nc.sync.dma_start(out=b_t[:], in_=b_dram.ap().rearrange("(j p) -> p j", p=128))
            bv_bcast = consts.tile([128, FDIM], f32, tag="bv_bcast", name="bv_bcast")
            nc.sync.dma_start(
                out=bv_bcast[:],
                in_=bv.ap().rearrange("(a f) -> a f", a=1).to_broadcast([128, FDIM]))

            # ---- persistent activation tiles ----
            qT_sb = [qt_pool.tile([128, T], bf16, tag=f"qT{j}", name=f"qT{j}") for j in range(NFT)]
            kT_sb = [kt_pool.tile([128, T], bf16, tag=f"kT{j}", name=f"kT{j}") for j in range(NFT)]
            vaug = [vaug_pool.tile([128, H * (DK + 1)], bf16, tag=f"va{tt}", name=f"va{tt}")
                    for tt in range(NCH)]
            # ones column per head in V_aug
            for tt in range(NCH):
                va = vaug[tt][:].rearrange("p (h d) -> p h d", d=DK + 1)
                nc.vector.memset(va[:, :, DK:DK + 1], 1.0)

            # o2[j]: rows 0-63 head 2j, rows 64-127 head 2j+1; cols = q
            o2_sb = [osb_pool.tile([128, T], bf16, tag=f"o2_{j}", name=f"o2_{j}")
                     for j in range(NFT)]

            # ============ PSUM (8 banks) ============================
            #  tag "s":  2 x [128,1024] f32 = 4 banks (scores; also
            #            borrowed by QKV projections and P3 y-tiles)
            #  tag "oA"/"oB": 2 x [65,512] each = 4 banks (attnV accum)
            with (
                tc.tile_pool(name="xt", bufs=12) as xt_pool,
                tc.tile_pool(name="mask", bufs=16) as mask_pool,
                tc.tile_pool(name="praw", bufs=3) as praw_pool,
                tc.tile_pool(name="pm", bufs=3) as pm_pool,
                tc.tile_pool(name="epi", bufs=1) as epi_pool,
                tc.tile_pool(name="ysb", bufs=1) as ysb_pool,
                tc.tile_pool(name="psum", bufs=2, space="PSUM") as psum_pool,
            ):
                def alloc_xT(tag):
                    # all 4 feature tiles in one [128, 4*T] tile: x_big[:,
                    # fc*T + t] = x^T[fc*128 + p, t]
                    return xt_pool.tile([128, 4 * T], bf16, tag=tag, bufs=1,
                                        name="xt")

                def load_xT_half(xT_dram, big, h):
                    # one t-column-half of all 4 feature tiles in ONE DMA
                    sl = slice(h * 1024, (h + 1) * 1024)
                    nc.sync.dma_start(
                        out=big[:].rearrange("p (f t) -> p f t", t=T)[:, :, sl],
                        in_=xT_dram.ap()[:, sl].rearrange("(f p) t -> p f t", p=128))

                def v_proj_tile(tt):
                    ps = psum_pool.tile([128, 512], f32, tag="s", name="vp")
                    for fc in range(4):
                        nc.tensor.matmul(
                            ps[:],
                            xts_v[:, fc * T + tt * 128:fc * T + (tt + 1) * 128],
                            wv_sb[:, fc * FDIM:(fc + 1) * FDIM],
                            start=(fc == 0), stop=(fc == 3),
                        )
                    va = vaug[tt][:].rearrange("p (h d) -> p h d", d=DK + 1)
                    nc.vector.scalar_tensor_tensor(
                        out=va[:, :, 0:DK],
                        in0=ps[:].rearrange("p (h d) -> p h d", d=DK),
                        scalar=1.0,
                        in1=bv_bcast[:].rearrange("p (h d) -> p h d", d=DK),
                        op0=Alu.mult, op1=Alu.add,
                    )

                def proj_group(j, xts, w_sb, b_t, dst, s):
                    ps = psum_pool.tile([128, 512], f32, tag="s", name="qkp")
                    for fc in range(4):
                        nc.tensor.matmul(
                            ps[:],
                            w_sb[:, fc * FDIM + j * 128:fc * FDIM + (j + 1) * 128],
                            xts[:, fc * T + s * 512:fc * T + (s + 1) * 512],
                            start=(fc == 0), stop=(fc == 3),
                        )
                    nc.vector.tensor_scalar_add(
                        dst[:, s * 512:(s + 1) * 512], ps[:], b_t[:, j:j + 1])

                def p3_unit(qb, i):
                    qsl = slice(qb * QBS, (qb + 1) * QBS)
                    y_ps = psum_pool.tile([128, 512], f32, tag="s", name="y")
                    for j in range(NFT):
                        nc.tensor.matmul(
                            y_ps[:],
                            wo_sb[:, j * FDIM + i * 128:j * FDIM + (i + 1) * 128],
                            o2_sb[j][:, qsl],
                            start=(j == 0), stop=(j == NFT - 1),
                        )
                    y_sb = ysb_pool.tile([128, 512], f32, tag="ysb", bufs=2, name="ysb")
                    nc.vector.tensor_scalar_add(y_sb[:], y_ps[:], bo_sb[:, i:i + 1])
                    nc.sync.dma_start(out=yT[i * 128:(i + 1) * 128, qsl], in_=y_sb[:])

                # ---- P1 prefix ------------------------------------------
                # DMA order is the head critical path: wk, xk-s0, wq, xq-s0,
                # wv, xv-s0 and the first mask tiles come first; everything
                # else streams in behind while compute already runs.
                xts_k = alloc_xT("xk")
                xts_q = alloc_xT("xq")
                xts_v = alloc_xT("xv")

                mask_t = {}

                def mask_dma(qb, c):
                    # chunk mask duplicated 2x along free dim via a
                    # stride-0 source broadcast -> ONE DMA instruction
                    mt = mask_pool.tile([128, 1024], bf16, tag=f"mk{c}", bufs=1,
                                        name="mask")
                    qsl = slice(qb * QBS, (qb + 1) * QBS)
                    src = (mbar.ap()[c, :, qsl]
                           .rearrange("p (a f) -> p a f", a=1)
                           .to_broadcast([128, 2, 512]))
                    nc.sync.dma_start(
                        out=mt[:].rearrange("p (a f) -> p a f", a=2), in_=src)
                    mask_t[c] = mt

                load_xT_half(xkT, xts_k, 0)
                load_xT_half(xqT, xts_q, 0)
                load_xT_half(xvT, xts_v, 0)
                mask_dma(0, 0)
                mask_dma(0, 1)
                mask_dma(0, 2)
                mask_dma(0, 3)
                load_xT_half(xkT, xts_k, 1)
                load_xT_half(xvT, xts_v, 1)
                for c in range(4, NCH):
                    mask_dma(0, c)
                load_xT_half(xqT, xts_q, 1)
                w_dma(wo_sb, woT)

                # q/k projection groups: K(j) group s covers t2 chunks
                # 4s..4s+3 (all needed by pair j of every q-block); Q(j)
                # group s is only needed once q-block s is reached.
                proj_k = lambda j, s: proj_group(j, xts_k, wk_sb, bk_sb, kT_sb[j], s)
                proj_q = lambda j, s: proj_group(j, xts_q, wq_sb, bq_sb, qT_sb[j], s)

                proj_k(0, 0)
                proj_q(0, 0)
                v_proj_tile(0)

                # extra-work schedule: (qb, pair) -> {c: [fns]}
                def extra_steps(qb, pair):
                    steps = {}
                    add = lambda c, fn: steps.setdefault(c, []).append(fn)
                    if qb == 0:
                        if pair == 0:
                            for c in range(3):
                                add(c, lambda s=c + 1: proj_k(0, s))
                            for c in range(15):
                                add(c, lambda tt=c + 1: v_proj_tile(tt))
                            for idx, c in enumerate((3, 6, 9, 12)):
                                add(c, lambda s=idx: proj_k(1, s))
                            add(10, lambda: proj_q(1, 0))
                        elif pair in (1, 2):
                            jj = pair + 1
                            for idx, c in enumerate((0, 4, 8, 12)):
                                add(c, lambda s=idx, j=jj: proj_k(j, s))
                            add(10, lambda j=jj: proj_q(j, 0))
                        else:
                            for j in range(4):
                                add(1 + 4 * j, lambda j=j: proj_q(j, 1))
                    else:
                        # one P3 unit of the previous q-block per pair, plus
                        # the Q projection slice needed two q-blocks ahead
                        add(8, lambda i=pair, q=qb - 1: p3_unit(q, i))
                        if qb < 3:
                            add(3, lambda j=pair, s=qb + 1: proj_q(j, s))
                    return steps

                def epilogue(qb, pair, oA, oB):
                    pp = qb * 4 + pair
                    qsl = slice(qb * QBS, (qb + 1) * QBS)
                    # DVE lanes cannot move data across partitions: copy the
                    # denominator rows (partition 64) into an aligned SBUF
                    # tile, then DMA does the partition reshapes/broadcasts.
                    dn = epi_pool.tile([65, 1024], f32, tag="dn", bufs=2, name="dn")
                    nc.vector.tensor_copy(dn[64:65, 0:512], oA[64:65, :])
                    nc.vector.tensor_copy(dn[64:65, 512:1024], oB[64:65, :])
                    # [1,1024] -> [64,16]: partitions 0-31 head A, 32-63 head B
                    rbs = epi_pool.tile([64, 16], f32, tag="rbs", bufs=2, name="rbs")
                    nc.sync.dma_start(out=rbs[:], in_=dn[64:65, :])
                    rbr = epi_pool.tile([64, 16], f32, tag="rbr", bufs=2, name="rbr")
                    nc.vector.reciprocal(rbr[:], rbs[:])
                    rows = rscratch.ap()[2 * pp:2 * pp + 2, :]
                    nc.sync.dma_start(out=rows, in_=rbr[:])
                    # broadcast both rows back across 64 partitions in one DMA:
                    # rb cols 0-511 = 1/denomA, 512-1023 = 1/denomB
                    rb = epi_pool.tile([64, 1024], f32, tag="rb", bufs=2, name="rb")
                    nc.sync.dma_start(
                        out=rb[:].rearrange("p (r f) -> p r f", r=2),
                        in_=rows.rearrange("(a r) f -> a r f", a=1)
                                .to_broadcast([64, 2, 512]))
                    # head A lands on partitions 0-63 directly; head B needs a
                    # DMA partition-shift to o2 rows 64-127
                    nc.vector.tensor_mul(o2_sb[pair][0:64, qsl], oA[0:64, :],
                                         rb[:, 0:512])
                    osmB = epi_pool.tile([64, 512], bf16, tag="osm", bufs=2, name="osm")
                    nc.vector.tensor_mul(osmB[:], oB[0:64, :], rb[:, 512:1024])
                    nc.sync.dma_start(out=o2_sb[pair][64:128, qsl], in_=osmB[:])

                # ---- P2 main loop ----
                SKEW = 3          # attnV trails scores/exp by 3 chunks so a
                                  # slow (GpSimd) mask-mul never head-of-line
                                  # blocks the PE queue
                GP_CHUNKS = (3, 8, 13)
                for qb in range(NQB):
                    qsl = slice(qb * QBS, (qb + 1) * QBS)
                    for pair in range(NQB):
                        hA, hB = 2 * pair, 2 * pair + 1
                        steps = extra_steps(qb, pair)
                        oA = psum_pool.tile([DK + 1, 512], f32, tag="oA",
                                            bufs=2, name="oA")
                        oB = psum_pool.tile([DK + 1, 512], f32, tag="oB",
                                            bufs=2, name="oB")
                        p_ms = {}
                        for c in range(NCH + SKEW):
                            if c < NCH:
                                s_t = psum_pool.tile([128, 1024], f32, tag="s",
                                                     name="s")
                                nc.tensor.matmul(
                                    s_t[:, 0:512],
                                    kT_sb[pair][0:64, c * 128:(c + 1) * 128],
                                    qT_sb[pair][0:64, qsl],
                                    start=True, stop=True,
                                )
                                nc.tensor.matmul(
                                    s_t[:, 512:1024],
                                    kT_sb[pair][64:128, c * 128:(c + 1) * 128],
                                    qT_sb[pair][64:128, qsl],
                                    start=True, stop=True,
                                )
                                p_raw = praw_pool.tile([128, 1024], bf16,
                                                       tag="praw", bufs=4,
                                                       name="praw")
                                nc.scalar.activation(p_raw[:], s_t[:], Exp,
                                                     bias=0.0, scale=0.125)
                                p_m = pm_pool.tile([128, 1024], bf16, tag="pm",
                                                   bufs=5, name="pm")
                                eng = nc.gpsimd if c in GP_CHUNKS else nc.vector
                                eng.tensor_mul(p_m[:], p_raw[:], mask_t[c][:])
                                p_ms[c] = p_m
                            if c >= SKEW:
                                cc = c - SKEW
                                nc.tensor.matmul(
                                    oA[:],
                                    vaug[cc][:, hA * (DK + 1):(hA + 1) * (DK + 1)],
                                    p_ms[cc][:, 0:512],
                                    start=(cc == 0), stop=(cc == NCH - 1),
                                )
                                nc.tensor.matmul(
                                    oB[:],
                                    vaug[cc][:, hB * (DK + 1):(hB + 1) * (DK + 1)],
                                    p_ms[cc][:, 512:1024],
                                    start=(cc == 0), stop=(cc == NCH - 1),
                                )
                                del p_ms[cc]
                            for fn in steps.get(c, ()):
                                fn()
                            # refresh one mask tile per iteration during the
                            # last pair (spread out the DMA burst)
                            if pair == 3 and qb < NQB - 1 and c < NCH:
                                mask_dma(qb + 1, c)
                        epilogue(qb, pair, oA, oB)

                # final output projection for the last q-block
                for i in range(NFT):
                    p3_unit(NQB - 1, i)

    nc.compile()
    return nc


def _get_nc():
    if "nc" not in _cache:
        _cache["nc"] = _build_nc()
    return _cache["nc"]


def _make_in_maps(inputs):
    query = np.asarray(inputs["query"], np.float32)
    key = np.asarray(inputs["key"], np.float32)
    value = np.asarray(inputs["value"], np.float32)
    mask = np.asarray(inputs["mask"], bool)
    shared = {
        "wqT": np.ascontiguousarray(np.asarray(inputs["Wq"], np.float32).T).astype(BF16),
        "wkT": np.ascontiguousarray(np.asarray(inputs["Wk"], np.float32).T).astype(BF16),
        "wvT": np.ascontiguousarray(np.asarray(inputs["Wv"], np.float32).T).astype(BF16),
        "woT": np.ascontiguousarray(np.asarray(inputs["Wo"], np.float32).T).astype(BF16),
        "bq": np.asarray(inputs["bq"], np.float32),
        "bk": np.asarray(inputs["bk"], np.float32),
        "bv": np.asarray(inputs["bv"], np.float32),
        "bo": np.asarray(inputs["bo"], np.float32),
    }
    in_maps = []
    for b in range(N_CORES):
        m = dict(shared)
        m["xqT"] = np.ascontiguousarray(query[b].T).astype(BF16)
        m["xkT"] = np.ascontiguousarray(key[b].T).astype(BF16)
        m["xvT"] = np.ascontiguousarray(value[b].T).astype(BF16)
        mb = (~mask[b]).T.astype(BF16)          # (1 - mask)^T, [t2, q]
        m["mbar"] = np.ascontiguousarray(mb.reshape(NCH, 128, T))
        in_maps.append(m)
    return in_maps


def run(inputs, trace=False, **kwargs):
    from concourse.bass_utils import run_bass_kernel_spmd
    nc = _get_nc()
    res = run_bass_kernel_spmd(nc, _make_in_maps(inputs),
                               core_ids=list(range(N_CORES)),
                               trace=trace, **kwargs)
    y = np.stack([np.asarray(res.results[b]["yT"], np.float32).T
                  for b in range(N_CORES)])
    return y, res


def kernel(**inputs) -> np.ndarray:
    y, _ = run(inputs, trace=False)
    return y


# revision 17
# speedup vs baseline: 1.1878x; 1.1878x over previous
"""Multi-head attention (B=8, T=2048, D=512, H=8) on 8 TRN2 NeuronCores.

Sharding: data-parallel over batch - one batch element per core, no
collectives. Host-side prep: transpose x inputs to [D, T], cast matmul
operands to bf16, pass (1 - mask)^T chunk-major; transpose per-core y^T
back to [T, D].

Per-core algorithm (v2, "row-tiled transposed flash"):
  P1: Q^T = Wq x^T and K^T = Wk x^T, both as 4x [128, T] feature-major
      tiles (two heads per tile, NO padding).  V = x Wv^T with a ones
      column per head (denominator trick), chunk-major vaug tiles.
  P2: heads processed in PAIRS (2j, 2j+1).  Per (q-block of 512, pair,
      t2-chunk c):
        S_A^T = K_A^T.T @ Q_A^T   (K=64, PE rows 0-63)   } row-tiled,
        S_B^T = K_B^T.T @ Q_B^T   (K=64, PE rows 64-127) } concurrent
        both into ONE [128, 1024] PSUM tile (A: cols 0-511, B: 512-1023)
        P_raw = exp(S/8)          one ACTIVATE, N=1024
        P     = P_raw * maskbar   one tensor_mul vs a duplicated mask
                                  tile (1/3 of chunks on GpSimd)
        O_aug^T += Vaug_h.T @ P   (M=65; row 64 = softmax denominator)
      epilogue per (qb, pair): denom rows -> split recip -> DRAM-bounce
      broadcast -> normalize straight into o2 (bf16).
  P3: y^T = Wo^T.T @ O^T (+bo) per q-block, interleaved into the next
      q-block's attention stream.

ScalarE exp (256 ACTIVATEs @ ~1.2us) is the roofline; everything else
is scheduled to hide underneath it.
"""

import numpy as np
import ml_dtypes

B, T, FDIM, H = 8, 2048, 512, 8
DK = FDIM // H          # 64
NFT = FDIM // 128       # 4 feature tiles
NCH = T // 128          # 16 t2-chunks
NQB = 4                 # q blocks
QBS = T // NQB          # 512
N_CORES = 8

BF16 = ml_dtypes.bfloat16

_cache = {}


def _build_nc():
    import concourse.bass as bass
    import concourse.mybir as mybir
    from concourse import bacc, tile

    f32 = mybir.dt.float32
    bf16 = mybir.dt.bfloat16
    Exp = mybir.ActivationFunctionType.Exp
    Alu = mybir.AluOpType

    nc = bacc.Bacc("TRN2", target_bir_lowering=False, debug=False,
                   num_devices=N_CORES)

    # DRAM I/O (per-core shard shapes)
    xqT = nc.dram_tensor("xqT", [FDIM, T], bf16, kind="ExternalInput")
    xkT = nc.dram_tensor("xkT", [FDIM, T], bf16, kind="ExternalInput")
    xvT = nc.dram_tensor("xvT", [FDIM, T], bf16, kind="ExternalInput")
    wqT = nc.dram_tensor("wqT", [FDIM, FDIM], bf16, kind="ExternalInput")
    wkT = nc.dram_tensor("wkT", [FDIM, FDIM], bf16, kind="ExternalInput")
    wvT = nc.dram_tensor("wvT", [FDIM, FDIM], bf16, kind="ExternalInput")
    woT = nc.dram_tensor("woT", [FDIM, FDIM], bf16, kind="ExternalInput")
    bq = nc.dram_tensor("bq", [FDIM], f32, kind="ExternalInput")
    bk = nc.dram_tensor("bk", [FDIM], f32, kind="ExternalInput")
    bv = nc.dram_tensor("bv", [FDIM], f32, kind="ExternalInput")
    bo = nc.dram_tensor("bo", [FDIM], f32, kind="ExternalInput")
    mbar = nc.dram_tensor("mbar", [NCH, 128, T], bf16, kind="ExternalInput")
    yT = nc.dram_tensor("yT", [FDIM, T], f32, kind="ExternalOutput")
    # DRAM bounce rows for partition-broadcasting softmax reciprocals
    rscratch = nc.dram_tensor("rscratch", [NQB * H, QBS], f32)

    with tile.TileContext(nc) as tc:
        with (
            tc.tile_pool(name="consts", bufs=1) as consts,
            tc.tile_pool(name="qt", bufs=1) as qt_pool,
            tc.tile_pool(name="kt", bufs=1) as kt_pool,
            tc.tile_pool(name="vaug", bufs=1) as vaug_pool,
            tc.tile_pool(name="osb", bufs=1) as osb_pool,
        ):
            # ---- consts: weights + biases ----
            # One [128, 4*512] tile per weight matrix -> ONE DMA instruction
            # each (the Sync engine issues descriptors at ~650ns apiece, so
            # instruction count, not bytes, sets the head latency).
            wq_sb = consts.tile([128, 4 * FDIM], bf16, tag="wq", name="wq")
            wk_sb = consts.tile([128, 4 * FDIM], bf16, tag="wk", name="wk")
            wv_sb = consts.tile([128, 4 * FDIM], bf16, tag="wv", name="wv")
            wo_sb = consts.tile([128, 4 * FDIM], bf16, tag="wo", name="wo")

            def w_dma(dst, src):
                nc.sync.dma_start(
                    out=dst[:].rearrange("p (f d) -> p f d", d=FDIM),
                    in_=src.ap().rearrange("(f p) d -> p f d", p=128))

            bq_sb = consts.tile([128, NFT], f32, tag="bq", name="bq")
            bk_sb = consts.tile([128, NFT], f32, tag="bk", name="bk")
            bo_sb = consts.tile([128, NFT], f32, tag="bo", name="bo")
            bv_bcast = consts.tile([128, FDIM], f32, tag="bv_bcast", name="bv_bcast")

            def small_consts_dma():
                for b_dram, b_t in ((bq, bq_sb), (bk, bk_sb), (bo, bo_sb)):
                    nc.sync.dma_start(out=b_t[:], in_=b_dram.ap().rearrange("(j p) -> p j", p=128))
                nc.sync.dma_start(
                    out=bv_bcast[:],
                    in_=bv.ap().rearrange("(a f) -> a f", a=1).to_broadcast([128, FDIM]))

            # ---- persistent activation tiles ----
            qT_sb = [qt_pool.tile([128, T], bf16, tag=f"qT{j}", name=f"qT{j}") for j in range(NFT)]
            kT_sb = [kt_pool.tile([128, T], bf16, tag=f"kT{j}", name=f"kT{j}") for j in range(NFT)]
            vaug = [vaug_pool.tile([128, H * (DK + 1)], bf16, tag=f"va{tt}", name=f"va{tt}")
                    for tt in range(NCH)]
            # ones column per head in V_aug
            for tt in range(NCH):
                va = vaug[tt][:].rearrange("p (h d) -> p h d", d=DK + 1)
                nc.vector.memset(va[:, :, DK:DK + 1], 1.0)

            # o2[j]: rows 0-63 head 2j, rows 64-127 head 2j+1; cols = q
            o2_sb = [osb_pool.tile([128, T], bf16, tag=f"o2_{j}", name=f"o2_{j}")
                     for j in range(NFT)]

            # ============ PSUM (8 banks) ============================
            #  tag "s":  2 x [128,1024] f32 = 4 banks (scores; also
            #            borrowed by QKV projections and P3 y-tiles)
            #  tag "oA"/"oB": 2 x [65,512] each = 4 banks (attnV accum)
            with (
                tc.tile_pool(name="xt", bufs=12) as xt_pool,
                tc.tile_pool(name="mask", bufs=16) as mask_pool,
                tc.tile_pool(name="praw", bufs=3) as praw_pool,
                tc.tile_pool(name="pm", bufs=3) as pm_pool,
                tc.tile_pool(name="epi", bufs=1) as epi_pool,
                tc.tile_pool(name="ysb", bufs=1) as ysb_pool,
                tc.tile_pool(name="psum", bufs=2, space="PSUM") as psum_pool,
            ):
                def alloc_xT(tag):
                    # all 4 feature tiles in one [128, 4*T] tile: x_big[:,
                    # fc*T + t] = x^T[fc*128 + p, t]
                    return xt_pool.tile([128, 4 * T], bf16, tag=tag, bufs=1,
                                        name="xt")

                def load_xT_half(xT_dram, big, h):
                    # one t-column-half of all 4 feature tiles in ONE DMA
                    sl = slice(h * 1024, (h + 1) * 1024)
                    nc.sync.dma_start(
                        out=big[:].rearrange("p (f t) -> p f t", t=T)[:, :, sl],
                        in_=xT_dram.ap()[:, sl].rearrange("(f p) t -> p f t", p=128))

                def v_proj_tile(tt):
                    ps = psum_pool.tile([128, 512], f32, tag="s", name="vp")
                    for fc in range(4):
                        nc.tensor.matmul(
                            ps[:],
                            xts_v[:, fc * T + tt * 128:fc * T + (tt + 1) * 128],
                            wv_sb[:, fc * FDIM:(fc + 1) * FDIM],
                            start=(fc == 0), stop=(fc == 3),
                        )
                    va = vaug[tt][:].rearrange("p (h d) -> p h d", d=DK + 1)
                    nc.vector.scalar_tensor_tensor(
                        out=va[:, :, 0:DK],
                        in0=ps[:].rearrange("p (h d) -> p h d", d=DK),
                        scalar=1.0,
                        in1=bv_bcast[:].rearrange("p (h d) -> p h d", d=DK),
                        op0=Alu.mult, op1=Alu.add,
                    )

                def proj_group(j, xts, w_sb, b_t, dst, s):
                    ps = psum_pool.tile([128, 512], f32, tag="s", name="qkp")
                    for fc in range(4):
                        nc.tensor.matmul(
                            ps[:],
                            w_sb[:, fc * FDIM + j * 128:fc * FDIM + (j + 1) * 128],
                            xts[:, fc * T + s * 512:fc * T + (s + 1) * 512],
                            start=(fc == 0), stop=(fc == 3),
                        )
                    nc.vector.tensor_scalar_add(
                        dst[:, s * 512:(s + 1) * 512], ps[:], b_t[:, j:j + 1])

                def p3_unit(qb, i):
                    qsl = slice(qb * QBS, (qb + 1) * QBS)
                    y_ps = psum_pool.tile([128, 512], f32, tag="s", name="y")
                    for j in range(NFT):
                        nc.tensor.matmul(
                            y_ps[:],
                            wo_sb[:, j * FDIM + i * 128:j * FDIM + (i + 1) * 128],
                            o2_sb[j][:, qsl],
                            start=(j == 0), stop=(j == NFT - 1),
                        )
                    y_sb = ysb_pool.tile([128, 512], f32, tag="ysb", bufs=2, name="ysb")
                    nc.vector.tensor_scalar_add(y_sb[:], y_ps[:], bo_sb[:, i:i + 1])
                    nc.sync.dma_start(out=yT[i * 128:(i + 1) * 128, qsl], in_=y_sb[:])

                # ---- P1 prefix ------------------------------------------
                # DMA order is the head critical path: wk, xk-s0, wq, xq-s0,
                # wv, xv-s0 and the first mask tiles come first; everything
                # else streams in behind while compute already runs.
                xts_k = alloc_xT("xk")
                xts_q = alloc_xT("xq")
                xts_v = alloc_xT("xv")

                mask_t = {}

                def mask_dma(qb, c):
                    # chunk mask duplicated into both head-halves (two plain
                    # DMAs; fancy stride-0 source APs cost ~2.5us to issue)
                    mt = mask_pool.tile([128, 1024], bf16, tag=f"mk{c}", bufs=1,
                                        name="mask")
                    qsl = slice(qb * QBS, (qb + 1) * QBS)
                    nc.sync.dma_start(out=mt[:, 0:512], in_=mbar[c, :, qsl])
                    nc.sync.dma_start(out=mt[:, 512:1024], in_=mbar[c, :, qsl])
                    mask_t[c] = mt

                # strict priority order on the serialized DMA-issue queue
                w_dma(wk_sb, wkT)
                load_xT_half(xkT, xts_k, 0)
                w_dma(wq_sb, wqT)
                load_xT_half(xqT, xts_q, 0)
                w_dma(wv_sb, wvT)
                load_xT_half(xvT, xts_v, 0)
                small_consts_dma()
                mask_dma(0, 0)
                mask_dma(0, 1)
                mask_dma(0, 2)
                mask_dma(0, 3)
                load_xT_half(xkT, xts_k, 1)
                load_xT_half(xvT, xts_v, 1)
                for c in range(4, NCH):
                    mask_dma(0, c)
                load_xT_half(xqT, xts_q, 1)
                w_dma(wo_sb, woT)

                # q/k projection groups: K(j) group s covers t2 chunks
                # 4s..4s+3 (all needed by pair j of every q-block); Q(j)
                # group s is only needed once q-block s is reached.
                proj_k = lambda j, s: proj_group(j, xts_k, wk_sb, bk_sb, kT_sb[j], s)
                proj_q = lambda j, s: proj_group(j, xts_q, wq_sb, bq_sb, qT_sb[j], s)

                proj_k(0, 0)
                proj_q(0, 0)
                v_proj_tile(0)

                # extra-work schedule: (qb, pair) -> {c: [fns]}
                def extra_steps(qb, pair):
                    steps = {}
                    add = lambda c, fn: steps.setdefault(c, []).append(fn)
                    if qb == 0:
                        if pair == 0:
                            for c in range(3):
                                add(c, lambda s=c + 1: proj_k(0, s))
                            for c in range(15):
                                add(c, lambda tt=c + 1: v_proj_tile(tt))
                            for idx, c in enumerate((3, 6, 9, 12)):
                                add(c, lambda s=idx: proj_k(1, s))
                            add(10, lambda: proj_q(1, 0))
                        elif pair in (1, 2):
                            jj = pair + 1
                            for idx, c in enumerate((0, 4, 8, 12)):
                                add(c, lambda s=idx, j=jj: proj_k(j, s))
                            add(10, lambda j=jj: proj_q(j, 0))
                        else:
                            for j in range(4):
                                add(1 + 4 * j, lambda j=j: proj_q(j, 1))
                    else:
                        # one P3 unit of the previous q-block per pair, plus
                        # the Q projection slice needed two q-blocks ahead
                        add(8, lambda i=pair, q=qb - 1: p3_unit(q, i))
                        if qb < 3:
                            add(3, lambda j=pair, s=qb + 1: proj_q(j, s))
                    return steps

                def epilogue(qb, pair, oA, oB):
                    pp = qb * 4 + pair
                    qsl = slice(qb * QBS, (qb + 1) * QBS)
                    # DVE lanes cannot move data across partitions: copy the
                    # denominator rows (partition 64) into an aligned SBUF
                    # tile, then DMA does the partition reshapes/broadcasts.
                    dn = epi_pool.tile([65, 1024], f32, tag="dn", bufs=2, name="dn")
                    nc.vector.tensor_copy(dn[64:65, 0:512], oA[64:65, :])
                    nc.vector.tensor_copy(dn[64:65, 512:1024], oB[64:65, :])
                    # [1,1024] -> [64,16]: partitions 0-31 head A, 32-63 head B
                    rbs = epi_pool.tile([64, 16], f32, tag="rbs", bufs=2, name="rbs")
                    nc.sync.dma_start(out=rbs[:], in_=dn[64:65, :])
                    rbr = epi_pool.tile([64, 16], f32, tag="rbr", bufs=2, name="rbr")
                    nc.vector.reciprocal(rbr[:], rbs[:])
                    rows = rscratch.ap()[2 * pp:2 * pp + 2, :]
                    nc.sync.dma_start(out=rows, in_=rbr[:])
                    # broadcast both rows back across 64 partitions in one DMA:
                    # rb cols 0-511 = 1/denomA, 512-1023 = 1/denomB
                    rb = epi_pool.tile([64, 1024], f32, tag="rb", bufs=2, name="rb")
                    nc.sync.dma_start(
                        out=rb[:].rearrange("p (r f) -> p r f", r=2),
                        in_=rows.rearrange("(a r) f -> a r f", a=1)
                                .to_broadcast([64, 2, 512]))
                    # head A lands on partitions 0-63 directly; head B needs a
                    # DMA partition-shift to o2 rows 64-127
                    nc.vector.tensor_mul(o2_sb[pair][0:64, qsl], oA[0:64, :],
                                         rb[:, 0:512])
                    osmB = epi_pool.tile([64, 512], bf16, tag="osm", bufs=2, name="osm")
                    nc.vector.tensor_mul(osmB[:], oB[0:64, :], rb[:, 512:1024])
                    nc.sync.dma_start(out=o2_sb[pair][64:128, qsl], in_=osmB[:])

                # ---- P2 main loop ----
                SKEW = 3          # attnV trails scores/exp by 3 chunks so a
                                  # slow (GpSimd) mask-mul never head-of-line
                                  # blocks the PE queue
                GP_CHUNKS = (3, 8, 13)
                for qb in range(NQB):
                    qsl = slice(qb * QBS, (qb + 1) * QBS)
                    for pair in range(NQB):
                        last_pair = (qb == NQB - 1 and pair == NQB - 1)
                        # the very last pair drains with minimal skew and no
                        # GpSimd muls so the tail (epilogue + final P3) starts
                        # as soon as possible after the last exp
                        skew = 1 if last_pair else SKEW
                        gp_set = () if last_pair else GP_CHUNKS
                        hA, hB = 2 * pair, 2 * pair + 1
                        steps = extra_steps(qb, pair)
                        oA = psum_pool.tile([DK + 1, 512], f32, tag="oA",
                                            bufs=2, name="oA")
                        oB = psum_pool.tile([DK + 1, 512], f32, tag="oB",
                                            bufs=2, name="oB")
                        p_ms = {}
                        for c in range(NCH + skew):
                            if c < NCH:
                                s_t = psum_pool.tile([128, 1024], f32, tag="s",
                                                     name="s")
                                nc.tensor.matmul(
                                    s_t[:, 0:512],
                                    kT_sb[pair][0:64, c * 128:(c + 1) * 128],
                                    qT_sb[pair][0:64, qsl],
                                    start=True, stop=True,
                                )
                                nc.tensor.matmul(
                                    s_t[:, 512:1024],
                                    kT_sb[pair][64:128, c * 128:(c + 1) * 128],
                                    qT_sb[pair][64:128, qsl],
                                    start=True, stop=True,
                                )
                                p_raw = praw_pool.tile([128, 1024], bf16,
                                                       tag="praw", bufs=5,
                                                       name="praw")
                                nc.scalar.activation(p_raw[:], s_t[:], Exp,
                                                     bias=0.0, scale=0.125)
                                p_m = pm_pool.tile([128, 1024], bf16, tag="pm",
                                                   bufs=6, name="pm")
                                eng = nc.gpsimd if c in gp_set else nc.vector
                                eng.tensor_mul(p_m[:], p_raw[:], mask_t[c][:])
                                p_ms[c] = p_m
                            if c >= skew:
                                cc = c - skew
                                nc.tensor.matmul(
                                    oA[:],
                                    vaug[cc][:, hA * (DK + 1):(hA + 1) * (DK + 1)],
                                    p_ms[cc][:, 0:512],
                                    start=(cc == 0), stop=(cc == NCH - 1),
                                )
                                nc.tensor.matmul(
                                    oB[:],
                                    vaug[cc][:, hB * (DK + 1):(hB + 1) * (DK + 1)],
                                    p_ms[cc][:, 512:1024],
                                    start=(cc == 0), stop=(cc == NCH - 1),
                                )
                                del p_ms[cc]
                            for fn in steps.get(c, ()):
                                fn()
                            # refresh one mask tile per iteration during the
                            # last pair (spread out the DMA burst)
                            if pair == 3 and qb < NQB - 1 and c < NCH:
                                mask_dma(qb + 1, c)
                        epilogue(qb, pair, oA, oB)

                # final output projection for the last q-block
                for i in range(NFT):
                    p3_unit(NQB - 1, i)

    nc.compile()
    return nc


def _get_nc():
    if "nc" not in _cache:
        _cache["nc"] = _build_nc()
    return _cache["nc"]


def _make_in_maps(inputs):
    query = np.asarray(inputs["query"], np.float32)
    key = np.asarray(inputs["key"], np.float32)
    value = np.asarray(inputs["value"], np.float32)
    mask = np.asarray(inputs["mask"], bool)
    shared = {
        "wqT": np.ascontiguousarray(np.asarray(inputs["Wq"], np.float32).T).astype(BF16),
        "wkT": np.ascontiguousarray(np.asarray(inputs["Wk"], np.float32).T).astype(BF16),
        "wvT": np.ascontiguousarray(np.asarray(inputs["Wv"], np.float32).T).astype(BF16),
        "woT": np.ascontiguousarray(np.asarray(inputs["Wo"], np.float32).T).astype(BF16),
        "bq": np.asarray(inputs["bq"], np.float32),
        "bk": np.asarray(inputs["bk"], np.float32),
        "bv": np.asarray(inputs["bv"], np.float32),
        "bo": np.asarray(inputs["bo"], np.float32),
    }
    in_maps = []
    for b in range(N_CORES):
        m = dict(shared)
        m["xqT"] = np.ascontiguousarray(query[b].T).astype(BF16)
        m["xkT"] = np.ascontiguousarray(key[b].T).astype(BF16)
        m["xvT"] = np.ascontiguousarray(value[b].T).astype(BF16)
        mb = (~mask[b]).T.astype(BF16)          # (1 - mask)^T, [t2, q]
        m["mbar"] = np.ascontiguousarray(mb.reshape(NCH, 128, T))
        in_maps.append(m)
    return in_maps


def run(inputs, trace=False, **kwargs):
    from concourse.bass_utils import run_bass_kernel_spmd
    nc = _get_nc()
    res = run_bass_kernel_spmd(nc, _make_in_maps(inputs),
                               core_ids=list(range(N_CORES)),
                               trace=trace, **kwargs)
    y = np.stack([np.asarray(res.results[b]["yT"], np.float32).T
                  for b in range(N_CORES)])
    return y, res


def kernel(**inputs) -> np.ndarray:
    y, _ = run(inputs, trace=False)
    return y
